# revision 11
# baseline (speedup 1.0000x reference)
"""Trainium2 Bass kernel for nn_MemoryMultiHeadedAttention.

Batch-sharded across 8 NeuronCores (1 batch element per core). Each core
computes logits, attn, new_cmem and an aux-loss partial sum; new_mem and
ae_loss are host-side trivia (new_mem == x since t == mem_len).

Per-core algorithm (all big matmuls fp32r = 1 cyc/row when moving >= 256):
  - transpose x/mem/cmem + weights once on PE -> i-on-partition layouts
  - qT = WqT.T@xT (pre-scaled by 1/8), kT, v natural (ones-augmented)
  - conv1d(mem) as accumulated matmuls directly in transposed layout
    (+ per-partition bias) -> compT -> new_cmem / ckT / cv_aug
  - per head: dotsT = k.qT -> exp on ACT (unnormalized; logits are bounded
    so no max subtraction needed) -> P'T; out_aug = v_aug.T @ P'T
    accumulated over j in two groups (cmem+x / mem) so the mem-only group
    doubles as the aux attention a1; a2 from compressed k/v; attn output
    recomputed in (i,j) layout and normalized in a single ACT pass via
    exp(dots - ln S') with a per-partition bias column.
  - logits = outT.T @ WoutT + bout
"""

import sys

sys.path.insert(0, "/opt/trn_rl_repo")

import numpy as np

import concourse.bass as bass
import concourse.tile as tile
from concourse import bacc, mybir
from concourse.bass_utils import run_bass_kernel_spmd
from concourse.masks import make_identity

F32 = mybir.dt.float32
F32R = mybir.dt.float32r
Exp = mybir.ActivationFunctionType.Exp
Ln = mybir.ActivationFunctionType.Ln
Square = mybir.ActivationFunctionType.Square
Copy = mybir.ActivationFunctionType.Copy

B = 8
H = 8
DH = 64
D = 512
T = 1024
ML = 1024
CM = 256
R = 4
J = CM + ML + T  # 2304
SCALE = DH ** -0.5
NCORES = 8
JT = J // 128  # 18
MEM_JT_LO, MEM_JT_HI = CM // 128, (CM + ML) // 128  # j-tiles 2..9 = mem region


def _bcast_row(ap, nparts):
    """Broadcast a row AP (last dim = N) to (nparts, N) with 0 partition stride."""
    return bass.AP(tensor=ap.tensor, offset=ap.offset,
                   ap=[[0, nparts], list(ap.ap[-1])])


def _build_nc():
    nc = bacc.Bacc("TRN2", target_bir_lowering=False)

    x_d = nc.dram_tensor("x", [T, D], F32, kind="ExternalInput")
    mem_d = nc.dram_tensor("mem", [ML, D], F32, kind="ExternalInput")
    cmem_d = nc.dram_tensor("cmem", [CM, D], F32, kind="ExternalInput")
    wq_d = nc.dram_tensor("Wq", [D, D], F32, kind="ExternalInput")
    wkv_d = nc.dram_tensor("Wkv", [2 * D, D], F32, kind="ExternalInput")
    wout_d = nc.dram_tensor("Wout", [D, D], F32, kind="ExternalInput")
    bout_d = nc.dram_tensor("bout", [D], F32, kind="ExternalInput")
    convw_d = nc.dram_tensor("conv_w", [D, D, R], F32, kind="ExternalInput")
    convb_d = nc.dram_tensor("conv_b", [D], F32, kind="ExternalInput")

    logits_d = nc.dram_tensor("logits", [T, D], F32, kind="ExternalOutput")
    attn_d = nc.dram_tensor("attn", [H, T, J], F32, kind="ExternalOutput")
    ncmem_d = nc.dram_tensor("new_cmem", [CM, D], F32, kind="ExternalOutput")
    aux_d = nc.dram_tensor("aux_sq", [1, 1], F32, kind="ExternalOutput")

    with tile.TileContext(nc) as tc:
        _emit(nc, tc, x_d, mem_d, cmem_d, wq_d, wkv_d, wout_d, bout_d,
              convw_d, convb_d, logits_d, attn_d, ncmem_d, aux_d)
    nc.compile()
    return nc


def _transpose_block(nc, psum_ap, src_ap, ident, start=True, stop=True):
    """psum_ap (128,128) <- src_ap.T via PE. start/stop follow PSUM
    zero-region rules: only the first write into a 2KB bank may set start."""
    nc.tensor.matmul(psum_ap, src_ap, ident, is_transpose=True,
                     start=start, stop=stop, skip_group_check=True)


def _emit(nc, tc, x_d, mem_d, cmem_d, wq_d, wkv_d, wout_d, bout_d,
          convw_d, convb_d, logits_d, attn_d, ncmem_d, aux_d):
    from contextlib import ExitStack

    ectx = ExitStack()
    with ectx:
        persist = ectx.enter_context(tc.tile_pool(name="persist", bufs=1))
        ident = persist.tile([128, 128], F32)
        make_identity(nc, ident)
        qT = persist.tile([128, 4, T], F32R)      # (o, t) head-packed, pre-scaled
        kT = persist.tile([128, 4, J], F32R)      # (o', j) head-packed
        v_aug = persist.tile([128, JT, H, DH + 1], F32R)
        ckT = persist.tile([128, 4, CM], F32R)
        cv_aug = persist.tile([128, 2, H, DH + 1], F32R)
        convb_col = persist.tile([128, 4], F32)
        auxcols = persist.tile([64, H, 2], F32)
        ones64 = persist.tile([64, 1], F32)
        auxvec = persist.tile([64, 1], F32)
        onesb = persist.tile([1, 64], F32)
        onesf = persist.tile([128, 144], F32)

        nc.vector.memset(ones64, 1.0)
        nc.vector.memset(onesb, 1.0)
        nc.vector.memset(onesf, 1.0)
        nc.vector.tensor_copy(
            v_aug[:, :, :, DH:DH + 1],
            onesf.rearrange("p (a b c) -> p a b c", a=JT, b=H))
        nc.vector.tensor_copy(
            cv_aug[:, :, :, DH:DH + 1],
            onesf[:, 0:16].rearrange("p (a b c) -> p a b c", a=2, b=H))
        nc.sync.dma_start(out=convb_col,
                          in_=convb_d.ap().rearrange("(oct p) -> p oct", p=128))

        eng_ctr = [0]

        def evict(dst_ap, src_ap):
            """Alternate PSUM->SBUF evictions between DVE and ACT."""
            if eng_ctr[0] % 2 == 0:
                nc.vector.tensor_copy(dst_ap, src_ap)
            else:
                nc.scalar.copy(dst_ap, src_ap)
            eng_ctr[0] += 1

        # ================= projection / conv phase =================
        with tc.tile_pool(name="projB", bufs=1) as projB:
            kvT = projB.tile([128, 4, J], F32R)       # kv_input.T (i, j)
            wkvT = projB.tile([128, 4, 2 * D], F32R)  # Wkv.T (i, o')

            with tc.tile_pool(name="projA", bufs=1) as projA, \
                 tc.tile_pool(name="psP", bufs=4, space="PSUM") as psP:

                def load_transpose(src_rows_ap, n_tt, dst_sel):
                    """src (n_tt*128, 512) natural -> transposed into dst."""
                    for tt0 in range(0, n_tt, 4):
                        ntt = min(4, n_tt - tt0)
                        st = projA.tile([128, 4, D], F32, tag="stage", bufs=2)
                        nc.sync.dma_start(out=st[:, 0:ntt, :],
                                          in_=src_rows_ap[:, tt0:tt0 + ntt, :])
                        for ict in range(4):
                            pt = psP.tile([128, 512], F32, tag="tp")
                            for q in range(ntt):
                                _transpose_block(
                                    nc, pt[:, q * 128:(q + 1) * 128],
                                    st[:, q, ict * 128:(ict + 1) * 128], ident,
                                    start=(q == 0), stop=(q == ntt - 1))
                            dst_sel(ict, tt0 * 128, pt[:, 0:ntt * 128])

                def sel_into(dst, col_off):
                    def f(ict, rel, src):
                        n = src.shape[-1]
                        evict(dst[:, ict, col_off + rel:col_off + rel + n], src)
                    return f

                load_transpose(cmem_d.ap().rearrange("(tt p) i -> p tt i", p=128),
                               2, sel_into(kvT, 0))
                load_transpose(mem_d.ap().rearrange("(tt p) i -> p tt i", p=128),
                               8, sel_into(kvT, CM))
                load_transpose(x_d.ap().rearrange("(tt p) i -> p tt i", p=128),
                               8, sel_into(kvT, CM + ML))
                load_transpose(wkv_d.ap().rearrange("(tt p) i -> p tt i", p=128),
                               8, sel_into(wkvT, 0))

                # --- Wq -> wqT, then qT ---
                wqT = projA.tile([128, 4, D], F32R, tag="wtmp", bufs=2)
                load_transpose(wq_d.ap().rearrange("(tt p) i -> p tt i", p=128),
                               4, sel_into(wqT, 0))
                xT = kvT[:, :, CM + ML:J]
                for ot in range(4):
                    for c in range(2):
                        pq = psP.tile([128, 512], F32, tag="mm")
                        for ict in range(4):
                            nc.tensor.matmul(pq,
                                             wqT[:, ict, ot * 128:(ot + 1) * 128],
                                             xT[:, ict, c * 512:(c + 1) * 512],
                                             start=(ict == 0), stop=(ict == 3))
                        nc.scalar.activation(out=qT[:, ot, c * 512:(c + 1) * 512],
                                             in_=pq, func=Copy, scale=float(SCALE))
                # --- kT ---
                for ot in range(4):
                    for c in range(5):
                        w = 512 if c < 4 else 256
                        pk = psP.tile([128, 512], F32, tag="mm")
                        for ict in range(4):
                            nc.tensor.matmul(pk[:, 0:w],
                                             wkvT[:, ict, ot * 128:(ot + 1) * 128],
                                             kvT[:, ict, c * 512:c * 512 + w],
                                             start=(ict == 0), stop=(ict == 3))
                        evict(kT[:, ot, c * 512:c * 512 + w], pk[:, 0:w])
                # --- v natural -> v_aug ---
                for jt in range(JT):
                    pv = psP.tile([128, 512], F32, tag="mm")
                    for ict in range(4):
                        nc.tensor.matmul(pv,
                                         kvT[:, ict, jt * 128:(jt + 1) * 128],
                                         wkvT[:, ict, D:2 * D],
                                         start=(ict == 0), stop=(ict == 3))
                    evict(v_aug[:, jt, :, 0:DH], pv.rearrange("p (h d) -> p h d", d=DH))

                # --- conv: compT[oc, t4] = sum_{ic,k} conv_w[oc,ic,k] mem[4t4+k, ic] ---
                compT = projA.tile([128, 4, CM], F32R, tag="compT")
                memT = kvT[:, :, CM:CM + ML]
                for oct_ in range(4):
                    stw = projA.tile([128, D * R], F32, tag="stage", bufs=2)
                    nc.sync.dma_start(
                        out=stw,
                        in_=convw_d.ap().rearrange(
                            "(oct p) ic k -> p oct (ic k)", p=128)[:, oct_, :])
                    stw_k = stw.rearrange("p (ic k) -> p k ic", k=R)
                    cq = projA.tile([128, 4, R, 128], F32R, tag="wtmp", bufs=2)
                    for k in range(R):
                        for ict in range(4):
                            pt = psP.tile([128, 512], F32, tag="tp")
                            _transpose_block(nc, pt[:, 0:128],
                                             stw_k[:, k, ict * 128:(ict + 1) * 128],
                                             ident)
                            evict(cq[:, ict, k, :], pt[:, 0:128])
                    pc = psP.tile([128, 512], F32, tag="mm")
                    n = 0
                    for ict in range(4):
                        m_k = memT[:, ict, :].rearrange("p (t4 k) -> p k t4", k=R)
                        for k in range(R):
                            nc.tensor.matmul(pc[:, 0:CM],
                                             cq[:, ict, k, :],
                                             m_k[:, k, :],
                                             start=(n == 0), stop=(n == 15))
                            n += 1
                    nc.vector.tensor_scalar_add(compT[:, oct_, :], pc[:, 0:CM],
                                                convb_col[:, oct_:oct_ + 1])

                # new_cmem: transpose compT back to natural and store
                comp = projA.tile([128, 2, D], F32, tag="compnat")
                for tt4 in range(2):
                    pt = psP.tile([128, 512], F32, tag="tp")
                    for oct_ in range(4):
                        _transpose_block(
                            nc, pt[:, oct_ * 128:(oct_ + 1) * 128],
                            compT[:, oct_, tt4 * 128:(tt4 + 1) * 128].bitcast(F32),
                            ident, start=(oct_ == 0), stop=(oct_ == 3))
                    evict(comp[:, tt4, :], pt)
                    nc.sync.dma_start(out=ncmem_d.ap()[tt4 * 128:(tt4 + 1) * 128, :],
                                      in_=comp[:, tt4, :])
                # ckT (o', t4)
                for ot in range(4):
                    pk = psP.tile([128, 512], F32, tag="mm")
                    for oct_ in range(4):
                        nc.tensor.matmul(pk[:, 0:CM],
                                         wkvT[:, oct_, ot * 128:(ot + 1) * 128],
                                         compT[:, oct_, :],
                                         start=(oct_ == 0), stop=(oct_ == 3))
                    evict(ckT[:, ot, :], pk[:, 0:CM])
                # cv natural -> cv_aug
                for tt4 in range(2):
                    pv = psP.tile([128, 512], F32, tag="mm")
                    for oct_ in range(4):
                        nc.tensor.matmul(pv,
                                         compT[:, oct_, tt4 * 128:(tt4 + 1) * 128],
                                         wkvT[:, oct_, D:2 * D],
                                         start=(oct_ == 0), stop=(oct_ == 3))
                    evict(cv_aug[:, tt4, :, 0:DH], pv.rearrange("p (h d) -> p h d", d=DH))

        # ================= attention + logits =================
        with tc.tile_pool(name="mid", bufs=1) as mid:
            outT = mid.tile([128, 4, T], F32R)  # normalized out.T head-packed

            with tc.tile_pool(name="attnP", bufs=3) as attnP, \
                 tc.tile_pool(name="repP", bufs=1) as repP, \
                 tc.tile_pool(name="smP", bufs=1) as smP, \
                 tc.tile_pool(name="psDT", bufs=2, space="PSUM") as psDT, \
                 tc.tile_pool(name="psG", bufs=1, space="PSUM") as psG:

                for h in range(H):
                    hp = (h % 2) * 64
                    ht = h // 2
                    qTh = qT[hp:hp + 64, ht, :]
                    kTh = kT[hp:hp + 64, ht, :]

                    # --- phase A: dotsT -> exp -> out/aux accumulation ---
                    G = {}
                    for g in range(2):
                        for c in range(2):
                            G[(g, c)] = psG.tile([65, 512], F32, tag=f"G{g}{c}", name=f"G_{h}_{g}_{c}")
                    for jt in range(JT):
                        grp = 1 if MEM_JT_LO <= jt < MEM_JT_HI else 0
                        pdT = psDT.tile([128, 1024], F32, tag="dT")
                        for c in range(2):
                            nc.tensor.matmul(pdT[:, c * 512:(c + 1) * 512],
                                             kTh[:, jt * 128:(jt + 1) * 128],
                                             qTh[:, c * 512:(c + 1) * 512],
                                             start=True, stop=True)
                        pt = attnP.tile([128, 1024], F32R, tag="PT")
                        nc.scalar.activation(out=pt, in_=pdT, func=Exp)
                        first = jt == (MEM_JT_LO if grp == 1 else 0)
                        last = jt == (MEM_JT_HI - 1 if grp == 1 else JT - 1)
                        for c in range(2):
                            nc.tensor.matmul(G[(grp, c)], v_aug[:, jt, h, :],
                                             pt[:, c * 512:(c + 1) * 512],
                                             start=first, stop=last)

                    # --- S rows for full softmax; evict outT (frees G0*) ---
                    cxrow = smP.tile([1, T], F32, tag="cxrow")
                    srow = smP.tile([1, T], F32, tag="srow")
                    s1row = smP.tile([1, T], F32, tag="s1row")
                    for c in range(2):
                        sl = slice(c * 512, (c + 1) * 512)
                        nc.vector.tensor_copy(cxrow[:, sl], G[(0, c)][64:65, :])
                        nc.vector.tensor_copy(s1row[:, sl], G[(1, c)][64:65, :])
                        nc.vector.tensor_add(srow[:, sl], G[(1, c)][64:65, :],
                                             cxrow[:, sl])
                    rS = repP.tile([64, T], F32, tag="rS")
                    r1 = repP.tile([64, T], F32, tag="r1")
                    for row, rep in ((srow, rS), (s1row, r1)):
                        prep = psDT.tile([128, 1024], F32, tag="dT",
                                         name=f"prep_{h}_{row.tensor.name}")
                        for c in range(2):
                            nc.tensor.matmul(prep[0:64, c * 512:(c + 1) * 512],
                                             onesb, row[0:1, c * 512:(c + 1) * 512],
                                             start=True, stop=True,
                                             skip_group_check=True)
                        nc.vector.reciprocal(rep, prep[0:64, :])

                    tcx = smP.tile([64, T], F32, tag="tcx")
                    for c in range(2):
                        sl = slice(c * 512, (c + 1) * 512)
                        nc.vector.tensor_copy(tcx[:, sl], G[(0, c)][0:64, :])
                        nc.vector.tensor_add(tcx[:, sl], tcx[:, sl],
                                             G[(1, c)][0:64, :])
                    if hp == 0:
                        nc.vector.tensor_mul(outT[0:64, ht, :], tcx, rS)
                    else:
                        odd = smP.tile([64, T], F32R, tag="odd")
                        nc.vector.tensor_mul(odd, tcx, rS)
                        nc.sync.dma_start(out=outT[64:128, ht, :], in_=odd)

                    # --- aux a1 partial (t1 = a1 while G1 lives) ---
                    t1 = smP.tile([64, T], F32, tag="t1")
                    for c in range(2):
                        sl = slice(c * 512, (c + 1) * 512)
                        nc.vector.tensor_mul(t1[:, sl], G[(1, c)][0:64, :], r1[:, sl])

                    # --- aux2: compressed-mem attention (reuses G0* slots) ---
                    G2 = [psG.tile([65, 512], F32, tag=f"G0{c}", name=f"G2_{h}_{c}")
                          for c in range(2)]
                    for jt2 in range(2):
                        pdT = psDT.tile([128, 1024], F32, tag="dT")
                        for c in range(2):
                            nc.tensor.matmul(pdT[:, c * 512:(c + 1) * 512],
                                             ckT[hp:hp + 64, ht,
                                                 jt2 * 128:(jt2 + 1) * 128],
                                             qTh[:, c * 512:(c + 1) * 512],
                                             start=True, stop=True)
                        pt = attnP.tile([128, 1024], F32R, tag="PT")
                        nc.scalar.activation(out=pt, in_=pdT, func=Exp)
                        for c in range(2):
                            nc.tensor.matmul(G2[c], cv_aug[:, jt2, h, :],
                                             pt[:, c * 512:(c + 1) * 512],
                                             start=(jt2 == 0), stop=(jt2 == 1))

                    s2row = smP.tile([1, T], F32, tag="s2row")
                    for c in range(2):
                        sl = slice(c * 512, (c + 1) * 512)
                        nc.vector.tensor_copy(s2row[:, sl], G2[c][64:65, :])
                    r2 = repP.tile([64, T], F32, tag="r2")
                    prep2 = psDT.tile([128, 1024], F32, tag="dT")
                    for c in range(2):
                        nc.tensor.matmul(prep2[0:64, c * 512:(c + 1) * 512],
                                         onesb, s2row[0:1, c * 512:(c + 1) * 512],
                                         start=True, stop=True,
                                         skip_group_check=True)
                    nc.vector.reciprocal(r2, prep2[0:64, :])

                    t2 = smP.tile([64, T], F32, tag="t2")
                    for c in range(2):
                        sl = slice(c * 512, (c + 1) * 512)
                        nc.vector.tensor_mul(t2[:, sl], G2[c][0:64, :], r2[:, sl])
                        nc.vector.tensor_sub(t2[:, sl], t1[:, sl], t2[:, sl])
                        dsq = smP.tile([64, 512], F32, tag="dsq")
                        nc.scalar.activation(out=dsq, in_=t2[:, sl], func=Square,
                                             accum_out=auxcols[:, h, c:c + 1])

                    # --- ln S' column for the attn-output bias ---
                    prow = psDT.tile([128, 1024], F32, tag="dT")
                    for it in range(8):
                        nc.tensor.matmul(prow[:, it:it + 1],
                                         srow[0:1, it * 128:(it + 1) * 128],
                                         ident[0:1, 0:1], is_transpose=True,
                                         start=(it == 0), stop=(it == 7),
                                         skip_group_check=True)
                    rScol = smP.tile([128, 8], F32, tag="rScol")
                    nc.vector.reciprocal(rScol, prow[:, 0:8])
                    negLnS = smP.tile([128, 8], F32, tag="negLnS")
                    nc.scalar.activation(out=negLnS, in_=rScol, func=Ln)

                    # --- phase B: attn output, single normalized exp pass ---
                    for it in range(8):
                        stage = attnP.tile([128, J], F32, tag="attnstage", bufs=2)
                        qTit = qTh[:, it * 128:(it + 1) * 128]
                        for c in range(2):
                            pb = psDT.tile([128, 1024], F32, tag="dT")
                            for u in range(2):
                                off = c * 1024 + u * 512
                                nc.tensor.matmul(pb[:, u * 512:(u + 1) * 512],
                                                 qTit, kTh[:, off:off + 512],
                                                 start=True, stop=True)
                            nc.scalar.activation(
                                out=stage[:, c * 1024:(c + 1) * 1024],
                                in_=pb, func=Exp, bias=negLnS[:, it:it + 1])
                        pb = psDT.tile([128, 1024], F32, tag="dT")
                        nc.tensor.matmul(pb[:, 0:256], qTit, kTh[:, 2048:J],
                                         start=True, stop=True)
                        nc.scalar.activation(out=stage[:, 2048:J],
                                             in_=pb[:, 0:256], func=Exp,
                                             bias=negLnS[:, it:it + 1])
                        nc.sync.dma_start(
                            out=attn_d.ap()[h, it * 128:(it + 1) * 128, :],
                            in_=stage)

                # --- aux reduce (fp32 matmul against ones: partition sum) ---
                nc.vector.tensor_reduce(auxvec,
                                        auxcols.rearrange("p h c -> p (h c)"),
                                        axis=mybir.AxisListType.X,
                                        op=mybir.AluOpType.add)
                pa = psDT.tile([128, 1024], F32, tag="dT")
                nc.tensor.matmul(pa[0:1, 0:1], auxvec, ones64,
                                 start=True, stop=True)
                auxsb = smP.tile([1, 1], F32, tag="auxsb")
                nc.vector.tensor_copy(auxsb, pa[0:1, 0:1])
                nc.sync.dma_start(out=aux_d.ap(), in_=auxsb)

            # ---------------- logits phase ----------------
            with tc.tile_pool(name="logP", bufs=1) as logP, \
                 tc.tile_pool(name="psL", bufs=4, space="PSUM") as psL:
                woutT = logP.tile([128, 4, D], F32R)
                bout_rep = logP.tile([128, D], F32)
                nc.gpsimd.dma_start(out=bout_rep, in_=_bcast_row(bout_d.ap(), 128))
                for tt0 in range(0, 4, 2):
                    stw = logP.tile([128, 2, D], F32, tag="stw", bufs=2)
                    nc.sync.dma_start(
                        out=stw,
                        in_=wout_d.ap().rearrange("(tt p) i -> p tt i", p=128)
                        [:, tt0:tt0 + 2, :])
                    for q in range(2):
                        for ict in range(4):
                            pt = psL.tile([128, 512], F32, tag="tp")
                            _transpose_block(nc, pt[:, 0:128],
                                             stw[:, q, ict * 128:(ict + 1) * 128],
                                             ident)
                            evict(woutT[:, ict, (tt0 + q) * 128:(tt0 + q + 1) * 128],
                                  pt[:, 0:128])
                for tt in range(8):
                    pl = psL.tile([128, 512], F32, tag="mm")
                    for ot in range(4):
                        nc.tensor.matmul(pl,
                                         outT[:, ot, tt * 128:(tt + 1) * 128],
                                         woutT[:, ot, :],
                                         start=(ot == 0), stop=(ot == 3))
                    lg = logP.tile([128, D], F32, tag="lg", bufs=2)
                    nc.vector.tensor_add(lg, pl, bout_rep)
                    nc.sync.dma_start(out=logits_d.ap()[tt * 128:(tt + 1) * 128, :],
                                      in_=lg)


_NC = None


def _get_nc():
    global _NC
    if _NC is None:
        _NC = _build_nc()
    return _NC


def _numpy_reference(x, mem, cmem, input_mask, Wq, Wkv, Wout, bout, conv_w, conv_b):
    """Pure-numpy fallback mirroring reference.py (only used if mask isn't all-True)."""
    b, t, d = x.shape
    mem_len, cmem_len = mem.shape[1], cmem.shape[1]

    def merge_heads(a):
        bb, n, _ = a.shape
        return a.reshape(bb, n, H, DH).transpose(0, 2, 1, 3)

    def full_attn(q, k, v):
        dots = np.einsum('bhid,bhjd->bhij', q, k) * (q.shape[-1] ** -0.5)
        dots = dots - dots.max(-1, keepdims=True)
        e = np.exp(dots)
        a = e / e.sum(-1, keepdims=True)
        return np.einsum('bhij,bhjd->bhid', a, v)

    q = x @ Wq.T
    kv_input = np.concatenate((cmem, mem, x), axis=1)
    kv = kv_input @ Wkv.T
    k, v = kv[..., :D], kv[..., D:]
    q, k, v = map(merge_heads, (q, k, v))
    dots = np.einsum('bhid,bhjd->bhij', q, k) * SCALE
    mask_value = -np.finfo(dots.dtype).max
    m = input_mask[:, None, :, None] & input_mask[:, None, None, :]
    m = np.pad(m, ((0, 0), (0, 0), (0, 0), (mem_len + cmem_len, 0)),
               constant_values=True)
    dots = np.where(m, dots, mask_value)
    dots = dots - dots.max(-1, keepdims=True)
    e = np.exp(dots)
    attn = e / e.sum(-1, keepdims=True)
    out = np.einsum('bhij,bhjd->bhid', attn, v)
    out = out.transpose(0, 2, 1, 3).reshape(b, t, d)
    logits = out @ Wout.T + bout

    queue = np.concatenate((mem, x), axis=1)
    old_mem = queue[:, :-ML]
    new_mem = queue[:, -ML:]
    om = old_mem.transpose(0, 2, 1)
    L = om.shape[2] // R
    # cm_[b, oc, t4] = sum_{ic,k} om[b, ic, 4*t4+k] * conv_w[oc, ic, k]
    cm_ = np.einsum('bikt,oik->bot',
                    om[:, :, :L * R].reshape(b, d, L, R).transpose(0, 1, 3, 2),
                    conv_w)
    compressed_mem = (cm_ + conv_b[None, :, None]).transpose(0, 2, 1)
    new_cmem = np.concatenate((cmem, compressed_mem), axis=1)[:, -CM:]

    ckv = compressed_mem @ Wkv.T
    ck, cv = ckv[..., :D], ckv[..., D:]
    ck, cv = map(merge_heads, (ck, cv))
    j_total = cmem_len + mem_len + t
    start = j_total - min(mem_len, ML) - T
    end = j_total - T
    ok, ov = k[:, :, start:end], v[:, :, start:end]
    aux = np.mean((full_attn(q, ok, ov) - full_attn(q, ck, cv)) ** 2)
    return (logits.astype(np.float32), new_mem.astype(np.float32),
            new_cmem.astype(np.float32),
            np.array([aux], np.float32), np.zeros(1, np.float32),
            attn.astype(np.float32))


def kernel(x, mem, cmem, input_mask, Wq, Wkv, Wout, bout, conv_w, conv_b):
    x = np.ascontiguousarray(np.asarray(x, np.float32))
    mem = np.ascontiguousarray(np.asarray(mem, np.float32))
    cmem = np.ascontiguousarray(np.asarray(cmem, np.float32))
    input_mask = np.asarray(input_mask)
    Wq = np.ascontiguousarray(np.asarray(Wq, np.float32))
    Wkv = np.ascontiguousarray(np.asarray(Wkv, np.float32))
    Wout = np.ascontiguousarray(np.asarray(Wout, np.float32))
    bout = np.ascontiguousarray(np.asarray(bout, np.float32))
    conv_w = np.ascontiguousarray(np.asarray(conv_w, np.float32))
    conv_b = np.ascontiguousarray(np.asarray(conv_b, np.float32))

    if not bool(input_mask.all()):
        return _numpy_reference(x, mem, cmem, input_mask, Wq, Wkv, Wout,
                                bout, conv_w, conv_b)

    nc = _get_nc()
    shared = {"Wq": Wq, "Wkv": Wkv, "Wout": Wout, "bout": bout,
              "conv_w": conv_w, "conv_b": conv_b}
    in_maps = [dict(shared, x=x[c], mem=mem[c], cmem=cmem[c])
               for c in range(NCORES)]
    res = run_bass_kernel_spmd(nc, in_maps, list(range(NCORES)))

    logits = np.stack([res.results[c]["logits"] for c in range(NCORES)])
    attn = np.stack([res.results[c]["attn"] for c in range(NCORES)])
    new_cmem = np.stack([res.results[c]["new_cmem"] for c in range(NCORES)])
    aux_total = float(sum(res.results[c]["aux_sq"][0, 0] for c in range(NCORES)))
    aux_loss = np.array([aux_total / (B * H * T * DH)], np.float32)
    new_mem = x.copy()
    ae_loss = np.zeros(1, np.float32)
    return (logits, new_mem, new_cmem, aux_loss, ae_loss, attn)


if __name__ == "__main__":
    _get_nc()
    print("built ok")
